# revision 1
# baseline (speedup 1.0000x reference)
"""NT-Xent loss on 8 Trainium2 NeuronCores.

Full inputs in, full (scalar) output out. Row-parallel sharding: core c
owns rows [1024c, 1024c+1024) of the 8192-row feature matrix and computes
its block of the similarity matrix against all columns. Inputs are
row-rotated per core so the single SPMD program sees its own rows at
local positions 0..1023 (static diagonal mask / positive-pair columns).

Per-core pipeline:
  normalize (DVE square -> TensorE ones-matmul colsum -> ACT exp(-ln/2))
  -> bf16 features nfT (d-major) -> bf16 matmul row-block x all-columns
  -> fused ACT exp(x/T) + row-sum accumulate -> lse;  positive term via
  elementwise mult + ones-matmul.  Host sums 8 per-core partials.
"""
import numpy as np
import ml_dtypes

import concourse.bass as bass  # noqa: F401
import concourse.tile as tile
import concourse.bacc as bacc_mod
from concourse import bacc, mybir
from concourse.bass_utils import run_bass_kernel_spmd
from concourse.hw_specs import get_activation_tables as _real_tables

B, D = 4096, 256
N = 2 * B                # 8192 rows/cols of sim matrix
NCORES = 8
RPC = N // NCORES        # 1024 rows per core
TEMP = 0.07
SCALE = 1.0 / TEMP
KG = 2                   # contraction groups: D = 256 = 2 * 128
CH = 2048                # column chunk (psum tile width)
NCH = N // CH            # 4 chunks
ST = 512                 # chunk-0 strip width
NST = CH // ST           # 4 strips in chunk 0
MT = RPC // 128          # 8 M-tiles per core
NEG = -1.0e9

AF = mybir.ActivationFunctionType
ALU = mybir.AluOpType
AX = mybir.AxisListType
f32 = mybir.dt.float32
bf16 = mybir.dt.bfloat16

_CACHE = {}


def _pinned_tables(arch):
    """Keep Exp/Ln only in natural_log_exp_and_others so the act-table
    insertion pass picks one set for the whole kernel (no reload thrash)."""
    tables = _real_tables(arch)
    out = {}
    for name, funcs in tables.items():
        if name != "natural_log_exp_and_others":
            funcs = {f for f in funcs if f.name not in ("Exp", "Ln")}
        out[name] = funcs
    return out


def _build_nc():
    bacc_mod.get_activation_tables = _pinned_tables
    nc = bacc.Bacc("TRN2", target_bir_lowering=False, debug=False,
                   enable_asserts=False, num_devices=NCORES,
                   num_swdge_queues=2)

    zt_d = nc.dram_tensor("zt", [KG, 128, N], f32, kind="ExternalInput")
    cones_d = nc.dram_tensor("cones", [128, 128], bf16, kind="ExternalInput")
    negid_d = nc.dram_tensor("negid", [128, 128], f32, kind="ExternalInput")
    vones_d = nc.dram_tensor("vones", [128, 1], f32, kind="ExternalInput")
    out_d = nc.dram_tensor("out", [1, 1], f32, kind="ExternalOutput")

    with tile.TileContext(nc) as tc:
        with (
            tc.tile_pool(name="singles", bufs=1) as singles,
            tc.tile_pool(name="nfp", bufs=1) as nfp,
            tc.tile_pool(name="ztp", bufs=2) as ztp,
            tc.tile_pool(name="sqp", bufs=2) as sqp,
            tc.tile_pool(name="invp", bufs=2) as invp,
            tc.tile_pool(name="expp", bufs=2) as expp,
            tc.tile_pool(name="ps", bufs=2, space="PSUM") as ps,
        ):
            # constants ride the SWDGE ring so feature loads own the SP ring
            cones = singles.tile([128, 128], bf16, tag="cones")
            nc.gpsimd.dma_start(out=cones, in_=cones_d.ap())
            negid = singles.tile([128, 128], f32, tag="negid")
            nc.gpsimd.dma_start(out=negid, in_=negid_d.ap())
            vones = singles.tile([128, 1], f32, tag="vones")
            nc.gpsimd.dma_start(out=vones, in_=vones_d.ap())

            # lse exp-sum accumulators: column m*NCH+g
            sums = singles.tile([128, MT * NCH], f32, tag="sums")

            # chunk 0 normalized features as 4 strip tiles; chunks 1..3 whole
            nf0 = [nfp.tile([128, KG, ST], bf16, tag=f"nf0s{s}",
                            name=f"nf0s{s}") for s in range(NST)]
            nfT = [None] + [nfp.tile([128, KG, CH], bf16, tag=f"nf{g}",
                                     name=f"nf{g}") for g in range(1, NCH)]

            zt_ap = zt_d.ap()

            def rhs_slice(g, n):
                """[128, KG-indexable] 512-wide rhs slice n of group g."""
                if g == 0:
                    return lambda kg: nf0[n][:, kg, :]
                return lambda kg: nfT[g][:, kg, 512 * n:512 * (n + 1)]

            def lhsT_slice(m):
                s, off = divmod(128 * m, ST)
                return lambda kg: nf0[s][:, kg, off:off + 128]

            def load(col0, width, tag, eng, bufs=1):
                """Issue the feature DMA for columns [col0, col0+width)."""
                zt_t = ztp.tile([128, KG, width], f32, tag=f"zt{tag}",
                                name=f"zt{tag}", bufs=bufs)
                eng.dma_start(
                    out=zt_t,
                    in_=zt_ap[:, :, col0:col0 + width].rearrange(
                        "k p c -> p k c"))
                return zt_t

            def normalize(dst, zt_t, width, tag):
                """Normalize a loaded zt tile into dst bf16 tile."""
                sq_t = sqp.tile([128, KG, width], bf16, tag=f"sq{tag}",
                                name=f"sq{tag}")
                nc.vector.tensor_mul(sq_t, zt_t, zt_t)
                nn_ps = ps.tile([128, width], f32, tag="ps", name="nn_ps")
                for n in range(width // 512):
                    for kg in range(KG):
                        nc.tensor.matmul(
                            nn_ps[:, 512 * n:512 * (n + 1)], cones,
                            sq_t[:, kg, 512 * n:512 * (n + 1)],
                            start=(kg == 0), stop=(kg == KG - 1))
                # 1/sqrt(nn) == exp(-0.5 * ln(nn))
                lnv = invp.tile([128, width], f32, tag=f"lnv{tag}",
                                name=f"lnv{tag}")
                nc.scalar.activation(lnv, nn_ps, AF.Ln)
                inv = invp.tile([128, width], f32, tag=f"inv{tag}",
                                name=f"inv{tag}")
                nc.scalar.activation(inv, lnv, AF.Exp, scale=-0.5)
                for kg in range(KG):
                    nc.vector.tensor_mul(dst[:, kg, :], zt_t[:, kg, :], inv)

            def main_group(g):
                for m in range(MT):
                    sim_ps = ps.tile([128, CH], f32, tag="ps", name="sim_ps")
                    for kg in range(KG):
                        lhsT = lhsT_slice(m)(kg)
                        for n in range(NST):
                            nc.tensor.matmul(
                                sim_ps[:, 512 * n:512 * (n + 1)], lhsT,
                                rhs_slice(g, n)(kg),
                                start=(kg == 0), stop=(kg == KG - 1),
                                skip_group_check=True)
                    if g == 0:
                        # mask self-similarity: row block m's diagonal is at
                        # columns [128m, 128m+128) of group 0
                        sl = sim_ps[:, 128 * m:128 * (m + 1)]
                        nc.vector.tensor_add(sl, sl, negid)
                    exp_sc = expp.tile([128, CH], bf16, tag="exp",
                                       name="exp_sc")
                    idx = m * NCH + g
                    nc.scalar.activation(exp_sc, sim_ps, AF.Exp, scale=SCALE,
                                         accum_out=sums[:, idx:idx + 1])

            # Issue all feature DMAs up front, spread over the three DMA
            # rings (SP / ACT-HWDGE / SWDGE) so transfers overlap: the
            # first main exp is gated on chunk 1 being loaded+normalized.
            zts = [load(ST * s, ST, "s", nc.sync, bufs=NST)
                   for s in range(NST)]
            ztc1 = load(CH, CH, "c1", nc.scalar)
            ztc2 = load(2 * CH, CH, "c2", nc.gpsimd)
            ztc3 = load(3 * CH, CH, "c3", nc.sync)

            # chunk 0 in fine strips (shortens the cold-start critical path);
            # normalize chunk g+1 BEFORE main group g so the PE never stalls
            # at a group boundary (keeps HAM warm).
            for s in range(NST):
                normalize(nf0[s], zts[s], ST, "s")
            normalize(nfT[1], ztc1, CH, "c")
            main_group(0)
            normalize(nfT[2], ztc2, CH, "c")
            main_group(1)
            # positive term: partner of local row i is local column i + 4096
            # (= column i of group 2). pos_i = <nf_i, nf_{i+4096}>.
            tmp_pos = sqp.tile([128, KG, RPC], bf16, tag="tpos")
            for kg in range(KG):
                nc.vector.tensor_mul(tmp_pos[:, kg, 0:ST],
                                     nf0[0][:, kg, :],
                                     nfT[2][:, kg, 0:ST])
                nc.vector.tensor_mul(tmp_pos[:, kg, ST:RPC],
                                     nf0[1][:, kg, :],
                                     nfT[2][:, kg, ST:RPC])
            pos_ps = ps.tile([128, RPC], f32, tag="ps")
            for n in range(RPC // 512):
                for kg in range(KG):
                    nc.tensor.matmul(
                        pos_ps[:, 512 * n:512 * (n + 1)], cones,
                        tmp_pos[:, kg, 512 * n:512 * (n + 1)],
                        start=(kg == 0), stop=(kg == KG - 1))
            fin = singles.tile([128, 2], f32, tag="fin")
            # pos_ps rows are identical (colsum replicated); reduce row-wise
            nc.vector.tensor_reduce(fin[:, 1:2], pos_ps, axis=AX.X,
                                    op=ALU.add)
            normalize(nfT[3], ztc3, CH, "c")
            main_group(2)
            main_group(3)

            # lse per row: ln(sum over the NCH group sums)
            rowsum = singles.tile([128, MT], f32, tag="rowsum")
            nc.vector.tensor_reduce(
                rowsum, sums.rearrange("p (m g) -> p m g", g=NCH),
                axis=AX.X, op=ALU.add)
            lse8 = singles.tile([128, MT], f32, tag="lse8")
            nc.scalar.activation(lse8, rowsum, AF.Ln)
            nc.vector.tensor_reduce(fin[:, 0:1], lse8, axis=AX.X, op=ALU.add)

            fin_ps = ps.tile([1, 2], f32, tag="ps")
            nc.tensor.matmul(fin_ps, vones, fin, start=True, stop=True)
            # fin_ps[0,0] = sum_p lse_p ; fin_ps[0,1] = 128 * sum_i pos_i
            possc = singles.tile([1, 1], f32, tag="possc")
            nc.vector.tensor_scalar_mul(possc, fin_ps[0:1, 1:2],
                                        SCALE / 128.0)
            outv = singles.tile([1, 1], f32, tag="outv")
            nc.vector.tensor_sub(outv, fin_ps[0:1, 0:1], possc)
            nc.sync.dma_start(out=out_d.ap(), in_=outv)

    nc.compile()
    return nc


def _get_nc():
    if "nc" not in _CACHE:
        _CACHE["nc"] = _build_nc()
    return _CACHE["nc"]


def _in_maps(z_i, z_j):
    feats = np.concatenate([np.asarray(z_i, dtype=np.float32),
                            np.asarray(z_j, dtype=np.float32)], axis=0)
    cones = np.ones((128, 128), dtype=ml_dtypes.bfloat16)
    negid = (NEG * np.eye(128)).astype(np.float32)
    vones = np.ones((128, 1), dtype=np.float32)
    maps = []
    for c in range(NCORES):
        zr = np.roll(feats, -RPC * c, axis=0)          # [N, D]
        zt = np.ascontiguousarray(zr.T).reshape(KG, 128, N)
        maps.append({"zt": zt, "cones": cones, "negid": negid,
                     "vones": vones})
    return maps


def kernel(z_i, z_j, _trace=False, _trace_kwargs=None):
    nc = _get_nc()
    maps = _in_maps(z_i, z_j)
    res = run_bass_kernel_spmd(nc, maps, core_ids=list(range(NCORES)),
                               trace=_trace, **(_trace_kwargs or {}))
    total = sum(float(res.results[c]["out"][0, 0]) for c in range(NCORES))
    out = np.array(np.float32(total / N))
    if _trace:
        kernel._last_result = res
    return out



# revision 2
# speedup vs baseline: 1.1434x; 1.1434x over previous
"""NT-Xent loss on 8 Trainium2 NeuronCores.

Full inputs in, full (scalar) output out. Row-parallel sharding: core c
owns rows [1024c, 1024c+1024) of the 8192-row feature matrix and computes
its block of the similarity matrix against all columns. Inputs are
row-rotated per core so the single SPMD program sees its own rows at
local positions 0..1023 (static diagonal mask / positive-pair columns).

v2 structure (ACT-engine-bound problem: 8.4M exps/core at 1 elem/cyc/lane):
  features staged bf16 on host (halves DMA, enables DVE 2x modes)
  -> normalize all 4 column chunks up front (DVE square -> PE ones-colsum
     -> ACT Ln -> ACT Exp(-ln/2) -> DVE mul), pipelined per chunk
  -> pos term once
  -> uninterrupted main stream: PE bf16 row-block matmuls ping-ponging two
     4-bank PSUM tiles while ACT runs back-to-back exp(x/T)+accumulate.
  Host sums 8 per-core partials.
"""
import numpy as np
import ml_dtypes

import concourse.bass as bass  # noqa: F401
import concourse.tile as tile
import concourse.bacc as bacc_mod
from concourse import bacc, mybir
from concourse.bass_utils import run_bass_kernel_spmd
from concourse.hw_specs import get_activation_tables as _real_tables

B, D = 4096, 256
N = 2 * B                # 8192 rows/cols of sim matrix
NCORES = 8
RPC = N // NCORES        # 1024 rows per core
TEMP = 0.07
SCALE = 1.0 / TEMP
KG = 2                   # contraction groups: D = 256 = 2 * 128
CH = 2048                # column chunk (psum tile width)
NCH = N // CH            # 4 chunks
MT = RPC // 128          # 8 M-tiles per core
NEG = -1.0e9

AF = mybir.ActivationFunctionType
ALU = mybir.AluOpType
AX = mybir.AxisListType
f32 = mybir.dt.float32
bf16 = mybir.dt.bfloat16

_CACHE = {}


def _pinned_tables(arch):
    """Keep Exp/Ln only in natural_log_exp_and_others so the act-table
    insertion pass picks one set for the whole kernel (no reload thrash)."""
    tables = _real_tables(arch)
    out = {}
    for name, funcs in tables.items():
        if name != "natural_log_exp_and_others":
            funcs = {f for f in funcs if f.name not in ("Exp", "Ln")}
        out[name] = funcs
    return out


def _build_nc():
    bacc_mod.get_activation_tables = _pinned_tables
    nc = bacc.Bacc("TRN2", target_bir_lowering=False, debug=False,
                   enable_asserts=False, num_devices=NCORES,
                   num_swdge_queues=2)

    ztb_d = nc.dram_tensor("ztb", [KG, 128, N], bf16, kind="ExternalInput")
    cones_d = nc.dram_tensor("cones", [128, 128], bf16, kind="ExternalInput")
    negid_d = nc.dram_tensor("negid", [128, 128], f32, kind="ExternalInput")
    vones_d = nc.dram_tensor("vones", [128, 1], f32, kind="ExternalInput")
    out_d = nc.dram_tensor("out", [1, 1], f32, kind="ExternalOutput")

    with tile.TileContext(nc) as tc:
        with (
            tc.tile_pool(name="singles", bufs=1) as singles,
            tc.tile_pool(name="nfp", bufs=1) as nfp,
            tc.tile_pool(name="sqp", bufs=2) as sqp,
            tc.tile_pool(name="invp", bufs=2) as invp,
            tc.tile_pool(name="expp", bufs=2) as expp,
            tc.tile_pool(name="ps", bufs=2, space="PSUM") as ps,
        ):
            # constants ride the SWDGE ring so feature loads own the HWDGEs
            cones = singles.tile([128, 128], bf16, tag="cones")
            nc.gpsimd.dma_start(out=cones, in_=cones_d.ap())
            negid = singles.tile([128, 128], f32, tag="negid")
            nc.gpsimd.dma_start(out=negid, in_=negid_d.ap())
            vones = singles.tile([128, 1], f32, tag="vones")
            nc.gpsimd.dma_start(out=vones, in_=vones_d.ap())

            # lse exp-sum accumulators: column m*NCH+g
            sums = singles.tile([128, MT * NCH], f32, tag="sums")

            ztb = [nfp.tile([128, KG, CH], bf16, tag=f"ztb{g}",
                            name=f"ztb{g}") for g in range(NCH)]
            nf = [nfp.tile([128, KG, CH], bf16, tag=f"nf{g}",
                           name=f"nf{g}") for g in range(NCH)]

            ztb_ap = ztb_d.ap()
            dma_engs = [nc.sync, nc.scalar, nc.gpsimd, nc.sync]
            for g in range(NCH):
                dma_engs[g].dma_start(
                    out=ztb[g],
                    in_=ztb_ap[:, :, CH * g:CH * (g + 1)].rearrange(
                        "k p c -> p k c"))

            def normalize(g):
                """ztb[g] (bf16) -> nf[g] = ztb[g] / ||col||."""
                sq_t = sqp.tile([128, KG, CH], bf16, tag="sq", name=f"sq{g}")
                nc.vector.tensor_mul(sq_t, ztb[g], ztb[g])
                nn_ps = ps.tile([128, CH], f32, tag="ps", name=f"nn{g}")
                for n in range(CH // 512):
                    for kg in range(KG):
                        nc.tensor.matmul(
                            nn_ps[:, 512 * n:512 * (n + 1)], cones,
                            sq_t[:, kg, 512 * n:512 * (n + 1)],
                            start=(kg == 0), stop=(kg == KG - 1))
                # 1/sqrt(nn) == exp(-0.5 * ln(nn))
                lnv = invp.tile([128, CH], f32, tag="lnv", name=f"lnv{g}")
                nc.scalar.activation(lnv, nn_ps, AF.Ln)
                inv = invp.tile([128, CH], bf16, tag="inv", name=f"inv{g}")
                nc.scalar.activation(inv, lnv, AF.Exp, scale=-0.5)
                for kg in range(KG):
                    nc.vector.tensor_mul(nf[g][:, kg, :], ztb[g][:, kg, :],
                                         inv)

            def main_group(g):
                for m in range(MT):
                    sim_ps = ps.tile([128, CH], f32, tag="ps", name="sim_ps")
                    for kg in range(KG):
                        lhsT = nf[0][:, kg, 128 * m:128 * (m + 1)]
                        for n in range(CH // 512):
                            nc.tensor.matmul(
                                sim_ps[:, 512 * n:512 * (n + 1)], lhsT,
                                nf[g][:, kg, 512 * n:512 * (n + 1)],
                                start=(kg == 0), stop=(kg == KG - 1),
                                skip_group_check=True)
                    if g == 0:
                        # mask self-similarity: row block m's diagonal is at
                        # columns [128m, 128m+128) of group 0
                        sl = sim_ps[:, 128 * m:128 * (m + 1)]
                        nc.vector.tensor_add(sl, sl, negid)
                    exp_sc = expp.tile([128, CH], bf16, tag="exp",
                                       name="exp_sc")
                    idx = m * NCH + g
                    nc.scalar.activation(exp_sc, sim_ps, AF.Exp, scale=SCALE,
                                         accum_out=sums[:, idx:idx + 1])

            for g in range(NCH):
                normalize(g)

            fin = singles.tile([128, 2], f32, tag="fin")
            main_group(0)

            # positive term: partner of local row i is local column i + 4096
            # (= column i of group 2). pos_i = <nf_i, nf_{i+4096}>.
            tmp_pos = sqp.tile([128, KG, RPC], bf16, tag="tpos")
            for kg in range(KG):
                nc.vector.tensor_mul(tmp_pos[:, kg, :],
                                     nf[0][:, kg, 0:RPC],
                                     nf[2][:, kg, 0:RPC])
            pos_ps = ps.tile([128, RPC], f32, tag="ps")
            for n in range(RPC // 512):
                for kg in range(KG):
                    nc.tensor.matmul(
                        pos_ps[:, 512 * n:512 * (n + 1)], cones,
                        tmp_pos[:, kg, 512 * n:512 * (n + 1)],
                        start=(kg == 0), stop=(kg == KG - 1))
            # pos_ps rows are identical (colsum replicated); reduce row-wise
            nc.vector.tensor_reduce(fin[:, 1:2], pos_ps, axis=AX.X,
                                    op=ALU.add)

            main_group(1)
            main_group(2)
            main_group(3)

            # lse per row: ln(sum over the NCH group sums)
            rowsum = singles.tile([128, MT], f32, tag="rowsum")
            nc.vector.tensor_reduce(
                rowsum, sums.rearrange("p (m g) -> p m g", g=NCH),
                axis=AX.X, op=ALU.add)
            lse8 = singles.tile([128, MT], f32, tag="lse8")
            nc.scalar.activation(lse8, rowsum, AF.Ln)
            nc.vector.tensor_reduce(fin[:, 0:1], lse8, axis=AX.X, op=ALU.add)

            fin_ps = ps.tile([1, 2], f32, tag="ps")
            nc.tensor.matmul(fin_ps, vones, fin, start=True, stop=True)
            # fin_ps[0,0] = sum_p lse_p ; fin_ps[0,1] = 128 * sum_i pos_i
            possc = singles.tile([1, 1], f32, tag="possc")
            nc.vector.tensor_scalar_mul(possc, fin_ps[0:1, 1:2],
                                        SCALE / 128.0)
            outv = singles.tile([1, 1], f32, tag="outv")
            nc.vector.tensor_sub(outv, fin_ps[0:1, 0:1], possc)
            nc.sync.dma_start(out=out_d.ap(), in_=outv)

    nc.compile()
    return nc


def _get_nc():
    if "nc" not in _CACHE:
        _CACHE["nc"] = _build_nc()
    return _CACHE["nc"]


def _in_maps(z_i, z_j):
    feats = np.concatenate([np.asarray(z_i, dtype=np.float32),
                            np.asarray(z_j, dtype=np.float32)], axis=0)
    cones = np.ones((128, 128), dtype=ml_dtypes.bfloat16)
    negid = (NEG * np.eye(128)).astype(np.float32)
    vones = np.ones((128, 1), dtype=np.float32)
    maps = []
    for c in range(NCORES):
        zr = np.roll(feats, -RPC * c, axis=0)          # [N, D]
        ztb = np.ascontiguousarray(zr.T).reshape(KG, 128, N).astype(
            ml_dtypes.bfloat16)
        maps.append({"ztb": ztb, "cones": cones, "negid": negid,
                     "vones": vones})
    return maps


def kernel(z_i, z_j, _trace=False, _trace_kwargs=None):
    nc = _get_nc()
    maps = _in_maps(z_i, z_j)
    res = run_bass_kernel_spmd(nc, maps, core_ids=list(range(NCORES)),
                               trace=_trace, **(_trace_kwargs or {}))
    total = sum(float(res.results[c]["out"][0, 0]) for c in range(NCORES))
    out = np.array(np.float32(total / N))
    if _trace:
        kernel._last_result = res
    return out
